# revision 12
# baseline (speedup 1.0000x reference)
"""Trainium2 Bass kernel for linear attention (elu+1 feature map).

Reference computation (B=4, N=M=8192, C=512, H=8, D=64):
    kv   = ref @ kv_w.T              -> k, v  [B,H,N,D]
    q    = tgt @ q_w.T               -> [B,H,M,D];  q,k -> elu(x)+1
    ctx  = sum_n k v^T per head      -> [B,H,D,D];  ksum = sum_n k
    x    = (q @ ctx) * SCALE / (1e-6 + q . ksum)
    out  = x @ proj_w.T + proj_b     -> [B,M,C]

Sharding: 8 cores = 4 batches x 2 row-halves. Each core computes partial
ctx/ksum from its half of N, pair-AllReduces the tiny per-head state, then
produces its half of M rows of the output.

Perf scheme (vs the bf16 baseline):
  - The three big projections (kv, q, out) run in fp8e4m3 with
    perf_mode=DoubleRow: operands packed [128, 2, F] so each matmul
    contracts 256 rows -> 2x PE throughput. Weights are host-scaled by
    64 to stay in e4m3 normal range; activations keep the 64x scale all
    the way through (it cancels exactly in x/denom, and the out-proj's
    net 4096x is divided out in the drain op).
  - elu+1 computed as min(exp(x/64 + ln64), 64) + max(x, 0) on the
    scaled pre-activations; sites alternate between an ACT-heavy (2 ACT
    + 1 DVE) and DVE-heavy (1 ACT + 2 DVE) variant to balance engines.
  - 1/denom via reciprocal_approx_fast (1 DVE op, ~51 ULP) instead of
    the exact iterative reciprocal (~6 cycles/elem); the 1e-6 guard is
    dropped (denom is strictly positive and O(1e3+)).
  - gpsimd (Pool) cannot touch PSUM and only supports copy/DMA ops,
    so elementwise work is spread over ACT and DVE only.
  - Output is written bf16 without bias; the host upcasts to f32 and
    adds proj_b.
"""

import math

import numpy as np
import ml_dtypes

import concourse.bass as bass
import concourse.mybir as mybir
from concourse import bacc
from concourse.tile import TileContext
from concourse.bass import ts
from concourse.bass_utils import run_bass_kernel_spmd

B, N, M, C, H = 4, 8192, 8192, 512, 8
D = C // H
SCALE = D**-0.5
NCORES = 8
BF = mybir.dt.bfloat16
F32 = mybir.dt.float32
F8 = mybir.dt.float8e4

WS = 64.0              # host weight scale (kv_w, q_w, proj_w)
LN_WS = math.log(WS)
PROJ_S = WS * WS       # net out-proj psum scale

_CACHE = {}


def build(R_ref, R_q, num_devices, replica_groups, lookahead=3):
    """Emit the SPMD graph. R_ref/R_q = rows of the ref/target shard."""
    P = 128
    KC = C // P          # 4 c-chunks
    KCP = KC // 2        # 2 c-pair-chunks (DoubleRow)
    NT1 = R_ref // P     # phase-1 row tiles
    CH = 512             # phase-2 chunk (columns of rows)
    NCH = R_q // CH      # phase-2 chunks
    RT = CH // P         # row tiles per chunk
    NPAIR = H // 2       # head pairs
    CP = C + NPAIR       # 516: 4 pairs x 129 cols (128 ctx + 1 ksum)
    STATE = P * CP       # collective payload floats
    DR = mybir.MatmulPerfMode.DoubleRow

    nc = bacc.Bacc("TRN2", target_bir_lowering=False, debug=False,
                   num_devices=num_devices)

    refT8 = nc.dram_tensor("refT8", [KCP, P, 2, R_ref], F8,
                           kind="ExternalInput")
    tgtT8 = nc.dram_tensor("tgtT8", [KCP, P, 2, R_q], F8,
                           kind="ExternalInput")
    kvwk8 = nc.dram_tensor("kvwk8", [KCP, P, 2, C], F8, kind="ExternalInput")
    kvwv8 = nc.dram_tensor("kvwv8", [KCP, P, 2, C], F8, kind="ExternalInput")
    qw8 = nc.dram_tensor("qw8", [KCP, P, 2, C], F8, kind="ExternalInput")
    pw8 = nc.dram_tensor("pw8", [KCP, P, 2, C], F8, kind="ExternalInput")
    E_const = nc.dram_tensor("E_const", [NPAIR, H, P], BF, kind="ExternalInput")
    out_ext = nc.dram_tensor("out", [R_q, C], BF, kind="ExternalOutput")
    cc_in = nc.dram_tensor("cc_in", [STATE], F32)
    cc_out = nc.dram_tensor("cc_out", [STATE], F32)

    def two(t):
        # [P, 2*F] tile -> [P, 2, F] view (DoubleRow operand layout)
        return t[:].rearrange("p (two f) -> p two f", two=2)

    with TileContext(nc) as tc:
        with (
            tc.tile_pool(name="res", bufs=1) as res,
            tc.tile_pool(name="mm", bufs=3, space="PSUM") as pmm,
            tc.tile_pool(name="kv", bufs=4) as kvp,
            tc.tile_pool(name="tmp", bufs=6) as tmp,
            tc.tile_pool(name="rc", bufs=3) as rcp,
            tc.tile_pool(name="qte", bufs=1) as qtep,
            tc.tile_pool(name="xt", bufs=2 * (1 + lookahead)) as xtp,
            tc.tile_pool(name="o", bufs=6) as op_,
        ):
            # ---- resident inputs ----
            # kv weights (k-half first) + refT pieces first so phase 1 can
            # start after ~0.6MB of DMA instead of the full ~6MB.
            NPIECE = 8
            PC_R = R_ref // NPIECE
            PC_Q = R_q // NPIECE
            kvwk_sb = []
            kvwv_sb = []
            for kcp in range(KCP):
                t = res.tile([P, 2 * C], F8, tag=f"kvwk{kcp}")
                nc.sync.dma_start(two(t), kvwk8[kcp])
                kvwk_sb.append(t)
            refT_sb = [res.tile([P, 2 * R_ref], F8, tag=f"refT{kcp}",
                                name=f"refT_sb{kcp}") for kcp in range(KCP)]
            for kcp in range(KCP):
                nc.sync.dma_start(two(refT_sb[kcp])[:, :, ts(0, PC_R)],
                                  refT8[kcp][:, :, ts(0, PC_R)])
            for kcp in range(KCP):
                t = res.tile([P, 2 * C], F8, tag=f"kvwv{kcp}")
                nc.sync.dma_start(two(t), kvwv8[kcp])
                kvwv_sb.append(t)
            for pc in range(1, NPIECE):
                for kcp in range(KCP):
                    nc.sync.dma_start(two(refT_sb[kcp])[:, :, ts(pc, PC_R)],
                                      refT8[kcp][:, :, ts(pc, PC_R)])
            qw_sb = []
            pw_sb = []
            for kcp in range(KCP):
                t = res.tile([P, 2 * C], F8, tag=f"qw{kcp}")
                nc.sync.dma_start(two(t), qw8[kcp])
                qw_sb.append(t)
                t = res.tile([P, 2 * C], F8, tag=f"pw{kcp}")
                nc.sync.dma_start(two(t), pw8[kcp])
                pw_sb.append(t)
            tgtT_sb = [res.tile([P, 2 * R_q], F8, tag=f"tgtT{kcp}",
                                name=f"tgtT_sb{kcp}") for kcp in range(KCP)]
            for pc in range(NPIECE):
                for kcp in range(KCP):
                    nc.sync.dma_start(two(tgtT_sb[kcp])[:, :, ts(pc, PC_Q)],
                                      tgtT8[kcp][:, :, ts(pc, PC_Q)])
            E_sb = []
            for p in range(NPAIR):
                e = res.tile([H, P], BF, tag=f"E{p}")
                nc.sync.dma_start(e[:], E_const[p])
                E_sb.append(e)

            # zero-init of cc-dependent tiles hoisted here: no dependency,
            # keeps the post-collective critical path to just the copies
            ctxs_bd = res.tile([P, C], BF, tag="ctxs_bd")
            nc.vector.memset(ctxs_bd[:], 0.0)
            Ksel = []
            for kc in range(KC):
                s = res.tile([P, H], BF, tag=f"Ksel{kc}", name=f"Ksel{kc}")
                nc.vector.memset(s[:], 0.0)
                Ksel.append(s)

            lnws = res.tile([P, 1], F32, tag="lnws")
            nc.vector.memset(lnws[:], LN_WS)

            # ---- elu(x/WS)+1 (times WS) on a scaled pre-activation ----
            # WS*elu1(x/WS) = min(WS*exp(x/WS), WS) + max(x,0).  The psum
            # reads (exp on ACT, max on DVE) are per-[128,512] psum tile;
            # the sbuf-side combine runs once per PAIR of sites at
            # [128,1024] to amortize the ~400ns fixed cost per DVE op.
            def elu_half(ps, e2, m2, half):
                sl = slice(half * 512, (half + 1) * 512)
                nc.scalar.activation(e2[:, sl], ps[:],
                                     mybir.ActivationFunctionType.Exp,
                                     scale=1.0 / WS, bias=lnws[:])
                nc.vector.tensor_scalar_max(m2[:, sl], ps[:], 0.0)

            def elu_combine(e2, m2, out_ap):
                nc.vector.scalar_tensor_tensor(
                    out_ap, e2[:], WS, m2[:],
                    mybir.AluOpType.min, mybir.AluOpType.add)

            # ---- phase 1: kv (fp8 DoubleRow), elu(k), ctx+ksum ----
            # v tiles are resident with a constant ones column per pair, so
            # each pair's ctx matmul also accumulates ksum (col 128); the
            # diagonal 64x64 blocks hold the two heads' ctx, off-diagonal
            # blocks are ignored garbage.  v keeps the 64x scale (ctx ends
            # up 4096x, ksum 64x; both cancel in x/denom).
            VN = 3
            v_res = [res.tile([P, CP], BF, tag=f"vres{r}", name=f"v_res{r}")
                     for r in range(VN)]
            for r in range(VN):
                ones_view = v_res[r][:].rearrange(
                    "p (g c) -> p g c", c=P + 1)[:, :, P : P + 1]
                nc.vector.memset(ones_view, 1.0)

            qte2 = [[None] * KCP for _ in range(NCH)]

            def qte_ap(j, mc):
                # [128, CH] view of channel-chunk mc of chunk j
                g, h = divmod(mc, 2)
                return qte2[j][g][:, h * CH : (h + 1) * CH]

            def qt_chunk(j):
                for g in range(KCP):
                    q2 = qtep.tile([P, 2 * CH], BF, tag=f"qte{j}_{g}",
                                   name=f"qte{j}_{g}")
                    e2 = tmp.tile([P, 2 * CH], BF, tag="ex")
                    m2 = tmp.tile([P, 2 * CH], BF, tag="mn")
                    qte2[j][g] = q2
                    for h in range(2):
                        mc = 2 * g + h
                        pq = pmm.tile([P, CH], F32, tag="mm")
                        for kcp in range(KCP):
                            nc.tensor.matmul(pq[:],
                                             two(qw_sb[kcp])[:, :, ts(mc, P)],
                                             two(tgtT_sb[kcp])[:, :,
                                                               ts(j, CH)],
                                             start=(kcp == 0),
                                             stop=(kcp == KCP - 1),
                                             perf_mode=DR)
                        elu_half(pq, e2, m2, h)
                    elu_combine(e2, m2, q2[:])

            pacc = tc.alloc_tile_pool(name="acc", bufs=1, space="PSUM")
            ctx_ps = [pacc.tile([P, P + 1], F32, tag=f"ctx{p}",
                                name=f"ctx_ps{p}") for p in range(NPAIR)]
            for i2 in range(NT1 // 2):
                k2 = kvp.tile([P, 2 * C], BF, tag="k")
                e2 = tmp.tile([P, 2 * C], BF, tag="ex")
                m2 = tmp.tile([P, 2 * C], BF, tag="mn")
                for half in range(2):
                    i = 2 * i2 + half
                    pk = pmm.tile([P, C], F32, tag="mm")
                    pv = pmm.tile([P, C], F32, tag="mm")
                    for kcp in range(KCP):
                        lhsT = two(refT_sb[kcp])[:, :, ts(i, P)]
                        nc.tensor.matmul(pk[:], lhsT, two(kvwk_sb[kcp]),
                                         start=(kcp == 0),
                                         stop=(kcp == KCP - 1),
                                         perf_mode=DR)
                        nc.tensor.matmul(pv[:], lhsT, two(kvwv_sb[kcp]),
                                         start=(kcp == 0),
                                         stop=(kcp == KCP - 1),
                                         perf_mode=DR)
                    elu_half(pk, e2, m2, half)
                    v_sb = v_res[i % VN]
                    v_view = v_sb[:].rearrange("p (g c) -> p g c",
                                               c=P + 1)[:, :, 0:P]
                    nc.scalar.activation(
                        v_view, pv[:].rearrange("p (g c) -> p g c", c=P),
                        mybir.ActivationFunctionType.Copy)
                elu_combine(e2, m2, k2[:])
                # ctx+ksum accumulate per head pair (one matmul each)
                for half in range(2):
                    i = 2 * i2 + half
                    v_sb = v_res[i % VN]
                    for p in range(NPAIR):
                        nc.tensor.matmul(
                            ctx_ps[p][:], k2[:, half * C + p * P
                                             : half * C + (p + 1) * P],
                            v_sb[:, p * (P + 1) : (p + 1) * (P + 1)],
                            start=(i == 0), stop=(i == NT1 - 1))

            # ---- collective: pair AllReduce of ctx + ksum ----
            # high_priority: the scheduler must drain ctx and launch the
            # AllReduce the moment phase 1 finishes, not after phase-2a's
            # queued elementwise work.
            with tc.high_priority():
                ctx_cp = res.tile([P, CP], F32, tag="ctx_cp")
                for p in range(NPAIR):
                    nc.scalar.activation(ctx_cp[:, ts(p, P + 1)], ctx_ps[p][:],
                                         mybir.ActivationFunctionType.Copy)
                nc.sync.dma_start(
                    cc_in[:].rearrange("(p f) -> p f", p=P), ctx_cp[:])
                nc.gpsimd.collective_compute(
                    "AllReduce", mybir.AluOpType.add,
                    replica_groups=replica_groups,
                    ins=[cc_in[:]], outs=[cc_out[:]])
            pacc.release()

            def build_state():
                # collective results -> ctxs_bd (block-diagonal pair blocks,
                # one matmul computes both heads' x) and Ksel columns
                with tc.high_priority():
                    ctxr = res.tile([P, CP], F32, tag="ctxr", name="ctxr")
                    nc.sync.dma_start(
                        ctxr[:], cc_out[:].rearrange("(p f) -> p f", p=P))
                    for p in range(NPAIR):
                        q0 = p * (P + 1)
                        nc.gpsimd.tensor_copy(ctxs_bd[0:D, p * P : p * P + D],
                                              ctxr[0:D, q0 : q0 + D])
                        nc.gpsimd.tensor_copy(
                            ctxs_bd[D:P, p * P + D : (p + 1) * P],
                            ctxr[D:P, q0 + D : q0 + P])
                    for kc in range(KC):
                        kq = kc * (P + 1) + P
                        nc.gpsimd.tensor_copy(
                            Ksel[kc][0:D, 2 * kc : 2 * kc + 1],
                            ctxr[0:D, kq : kq + 1])
                        nc.gpsimd.tensor_copy(
                            Ksel[kc][D:P, 2 * kc + 1 : 2 * kc + 2],
                            ctxr[D:P, kq : kq + 1])

            # ---- phase 2b: A(j) = denom/recip/x per chunk, B(j) = out-proj;
            # emitted with `lookahead` A-stages ahead of each B-stage so the
            # PE stream always has independent matmuls while DVE/ACT finish
            # the previous chunks.
            paux = tc.alloc_tile_pool(name="aux", bufs=1, space="PSUM")

            def stage_a(j):
                den = paux.tile([H, CH], F32, tag="rb", bufs=2, name="den")
                for kc in range(KC):
                    nc.tensor.matmul(den[:], Ksel[kc][:], qte_ap(j, kc),
                                     start=(kc == 0), stop=(kc == KC - 1))
                rec = rcp.tile([H, CH], F32, tag="rec")
                nc.vector.reciprocal_approx_fast(rec[:], den[:])
                recb = rcp.tile([H, CH], BF, tag="recb")
                nc.vector.tensor_scalar_mul(recb[:], rec[:], SCALE)
                # xt tiles hold two c-chunks each (DoubleRow lhsT layout for
                # the out-proj): xt01 = pairs 0,1; xt23 = pairs 2,3
                xts = [xtp.tile([P, 2 * CH], F8, tag=f"xt{g}",
                                name=f"xt{g}")
                       for g in range(2)]
                pxs = []
                for p in range(NPAIR):
                    px = paux.tile([P, CH], F32, tag="px", bufs=3,
                                   name="px")
                    nc.tensor.matmul(px[:], ctxs_bd[:, ts(p, P)],
                                     qte_ap(j, p), start=True, stop=True)
                    pxs.append(px)
                for p in range(NPAIR):
                    prb = paux.tile([P, CH], F32, tag="rb", bufs=2,
                                    name="prb")
                    nc.tensor.matmul(prb[:], E_sb[p][:], recb[:],
                                     start=True, stop=True)
                    rb = rcp.tile([P, CH], BF, tag="rbs")
                    nc.scalar.activation(rb[:], prb[:],
                                         mybir.ActivationFunctionType.Copy)
                    xt_view = two(xts[p // 2])[:, p % 2, :]
                    nc.vector.tensor_mul(xt_view, pxs[p][:], rb[:])
                return xts

            def stage_b(j, xts):
                for rt in range(RT):
                    po = pmm.tile([P, C], F32, tag="mm", name="po")
                    for g in range(2):
                        nc.tensor.matmul(po[:],
                                         two(xts[g])[:, :, ts(rt, P)],
                                         two(pw_sb[g]),
                                         start=(g == 0), stop=(g == 1),
                                         perf_mode=DR)
                    o_sb = op_.tile([P, C], BF, tag="o")
                    nc.scalar.activation(o_sb[:], po[:],
                                         mybir.ActivationFunctionType.Copy,
                                         scale=1.0 / PROJ_S)
                    nc.sync.dma_start(out_ext[ts(j * RT + rt, P), :], o_sb[:])

            for j in range(NCH):
                if j == max(NCH - 2, 0):
                    build_state()
                qt_chunk(j)
            pend = []
            for j in range(NCH):
                pend.append((j, stage_a(j)))
                if len(pend) > lookahead:
                    jj, xx = pend.pop(0)
                    stage_b(jj, xx)
            for jj, xx in pend:
                stage_b(jj, xx)
            paux.release()
    nc.compile()
    return nc


def _pack_pair(mat, scale=1.0, dtype=None):
    """[C, F] -> [KCP=2, 128, 2, F]: row r = kcp*256 + po*128 + pi goes to
    [kcp, pi, po, :]."""
    Crows, F = mat.shape
    assert Crows == C
    m = (np.asarray(mat, dtype=np.float32) * scale).reshape(2, 2, 128, F)
    m = np.ascontiguousarray(m.transpose(0, 2, 1, 3))  # [kcp, pi, po, F]
    return m.astype(dtype)


def _shard_inputs(target_data, reference_data, q_w, kv_w, proj_w, proj_b,
                  R, ncores):
    bf = ml_dtypes.bfloat16
    f8 = ml_dtypes.float8_e4m3
    kv_wT = np.ascontiguousarray(np.asarray(kv_w, dtype=np.float32).T)
    kvwk8 = _pack_pair(kv_wT[:, 0:C], WS, f8)
    kvwv8 = _pack_pair(kv_wT[:, C : 2 * C], WS, f8)
    qw8 = _pack_pair(np.ascontiguousarray(np.asarray(q_w).T), WS, f8)
    pw8 = _pack_pair(np.ascontiguousarray(np.asarray(proj_w).T), WS, f8)
    npair = H // 2
    E_const = np.zeros((npair, H, 128), dtype=bf)
    for p in range(npair):
        E_const[p, 2 * p, 0:D] = 1.0
        E_const[p, 2 * p + 1, D:128] = 1.0
    in_maps = []
    for c in range(ncores):
        b, half = divmod(c, 2)
        sl = slice(half * R, (half + 1) * R)
        refT = np.asarray(reference_data)[b, sl, :].T  # [C, R]
        tgtT = np.asarray(target_data)[b, sl, :].T
        in_maps.append({
            "refT8": _pack_pair(refT, 1.0, f8),
            "tgtT8": _pack_pair(tgtT, 1.0, f8),
            "kvwk8": kvwk8, "kvwv8": kvwv8, "qw8": qw8, "pw8": pw8,
            "E_const": E_const,
        })
    return in_maps


def kernel(target_data, reference_data, q_w, kv_w, proj_w, proj_b):
    R = M // 2
    key = (R, NCORES)
    if key not in _CACHE:
        _CACHE[key] = build(R, R, NCORES,
                            [[0, 1], [2, 3], [4, 5], [6, 7]], lookahead=3)
    nc = _CACHE[key]
    in_maps = _shard_inputs(target_data, reference_data, q_w, kv_w, proj_w,
                            proj_b, R, NCORES)
    res = run_bass_kernel_spmd(nc, in_maps, list(range(NCORES)))
    out = np.empty((B, M, C), dtype=np.float32)
    for c in range(NCORES):
        b, half = divmod(c, 2)
        out[b, half * R : (half + 1) * R, :] = np.asarray(
            res.results[c]["out"]).astype(np.float32)
    out += np.asarray(proj_b, dtype=np.float32)[None, None, :]
    return out
